# revision 1
# baseline (speedup 1.0000x reference)
"""Trainium2 Bass kernel for ContrastiveLoss (N=16384, D=1024, 8 NeuronCores).

Strategy (data-parallel over anchors):
  - Host shards rows across 8 cores: core i owns anchor rows [2048*i, 2048*(i+1)).
  - Host gathers pos/neg rows (gather commutes with row-wise normalization) and
    converts to fp16, so each core receives three contiguous [2048, 1024] fp16
    blocks (halves HBM traffic; fp16 keeps ~1e-5 relative accuracy here).
  - Device computes, per row r: sum(u*u), sum(u*v), sum(u*w) with a
    triple-buffered raw-Bass pipeline:
      ScalarE: Square+accum (row norm^2), Copy+accum (reduce of u*v product)
      VectorE: tensor_tensor mult fp16 2x mode (u*v, u*w), tensor_reduce (u*w)
      SP:      1MB HWDGE DMA loads, stats store
  - Row norms of pos/neg rows are gathers of the same global norm array, so
    the host epilogue (f64) reconstructs the reference math:
      ||a-b+eps||^2 = |a|^2 + |b|^2 + D*eps^2 - 2<a,b> (+ O(eps) sum terms,
      dropped: ~1e-8 relative), a = u/max(|u|,eps), then the margin loss.
"""

import sys

for _p in ("/opt/trn_rl_repo", "/root/.axon_site/_ro/trn_rl_repo"):
    if _p not in sys.path:
        sys.path.append(_p)

import numpy as np

N = 16384  # total rows
D = 1024  # embedding dim
NCORES = 8
RPC = N // NCORES  # rows per core = 2048
T = RPC // 128  # row-tiles per core = 16
G = 2  # row-tiles per DMA group (512 KB fp16 per load)
NG = T // G  # DMA groups per core = 4
BUFS = 5  # in-flight groups
EPS = 1e-6
MARGIN = 1.0

LAST_RESULT = None
_CACHE = {}


def _build_nc():
    import concourse.bass as bass
    import concourse.mybir as mybir

    f32 = mybir.dt.float32
    f16 = mybir.dt.float16
    nc = bass.Bass()
    anc = nc.declare_dram_parameter("anc", [RPC, D], f16, isOutput=False)
    pos = nc.declare_dram_parameter("pos", [RPC, D], f16, isOutput=False)
    neg = nc.declare_dram_parameter("neg", [RPC, D], f16, isOutput=False)
    out = nc.declare_dram_parameter("out", [3, 128, T], f32, isOutput=True)

    # DRAM row-tile t holds rows [128*t, 128*t+128); G tiles per DMA group.
    anc_r = anc[:, :].rearrange("(g a p) d -> g p a d", p=128, a=G)
    pos_r = pos[:, :].rearrange("(g a p) d -> g p a d", p=128, a=G)
    neg_r = neg[:, :].rearrange("(g a p) d -> g p a d", p=128, a=G)
    out_ap = out[:, :, :]

    Sq = mybir.ActivationFunctionType.Square
    Cp = mybir.ActivationFunctionType.Copy
    mult = mybir.AluOpType.mult
    add = mybir.AluOpType.add
    AX = mybir.AxisListType.X

    from contextlib import ExitStack

    with ExitStack() as ctx:
        sb = lambda nm, shape, dt: ctx.enter_context(nc.sbuf_tensor(nm, shape, dt))
        ps = lambda nm, shape, dt: ctx.enter_context(nc.psum_tensor(nm, shape, dt))
        sem = lambda nm: ctx.enter_context(nc.semaphore(nm))

        U = [sb(f"u{i}", [128, G, D], f16) for i in range(BUFS)]
        V = [sb(f"v{i}", [128, G, D], f16) for i in range(BUFS)]
        W = [sb(f"w{i}", [128, G, D], f16) for i in range(BUFS)]
        SQD = [ps(f"sqd{i}", [128, D], f32) for i in range(2)]  # ACT Square dumps
        CPD = [ps(f"cpd{i}", [128, D], f32) for i in range(2)]  # ACT Copy dumps
        S2 = [sb(f"s2{i}", [128, D], f16) for i in range(3)]  # DVE u*v product
        S3 = [sb(f"s3{i}", [128, D], f16) for i in range(3)]  # DVE u*w product
        nu2 = sb("nu2", [128, T], f32)
        dotp = sb("dotp", [128, T], f32)
        dotn = sb("dotn", [128, T], f32)
        # per-(tensor, slot) load sems: at most one outstanding DMA each,
        # so completion order is unambiguous
        SEM_U = [sem(f"sem_u{i}") for i in range(BUFS)]
        SEM_V = [sem(f"sem_v{i}") for i in range(BUFS)]
        SEM_W = [sem(f"sem_w{i}") for i in range(BUFS)]
        st_sem = sem("st_sem")  # +16 per completed store DMA
        # per-op-class retirement sems (count = ops retired); these give the
        # race detector an explicit edge for every buffer reuse
        dve_s2 = sem("dve_s2")  # DVE TT (u*v -> S2)
        dve_s3 = sem("dve_s3")  # DVE TT (u*w -> S3)
        dve_red = sem("dve_red")  # DVE reduce (S3 -> dotn col)
        act_sq = sem("act_sq")  # ACT Square (u -> nu2 col)
        act_s2 = sem("act_s2")  # ACT Copy (S2 -> dotp col, t%3!=0)
        block = ctx.enter_context(nc.Block())

        @block.sync
        def _(sync):
            for g in range(NG):
                if g >= BUFS:
                    m = G * (g - BUFS + 1)  # consumers of slot g-BUFS retired
                    sync.wait_ge(dve_s2, m)  # TT1 reads of U,V
                    sync.wait_ge(dve_s3, m)  # TT2 reads of U,W
                    sync.wait_ge(act_sq, m)  # Square reads of U
                b = g % BUFS
                sync.dma_start(out=U[b][:], in_=anc_r[g]).then_inc(SEM_U[b], 16)
                sync.dma_start(out=V[b][:], in_=pos_r[g]).then_inc(SEM_V[b], 16)
                sync.dma_start(out=W[b][:], in_=neg_r[g]).then_inc(SEM_W[b], 16)
            sync.wait_ge(act_sq, T)
            sync.wait_ge(dve_red, T)
            sync.wait_ge(act_s2, T)
            sync.dma_start(out=out_ap[0], in_=nu2[:]).then_inc(st_sem, 16)
            sync.dma_start(out=out_ap[1], in_=dotp[:]).then_inc(st_sem, 16)
            sync.dma_start(out=out_ap[2], in_=dotn[:]).then_inc(st_sem, 16)
            sync.wait_ge(st_sem, 48)

        @block.vector
        def _(vector):
            def reduces(t):
                # reduces for sub-tile t, issued one sub-tile late so the
                # producing TTs retired long before (no pipeline stall)
                vector.wait_ge(dve_s3, t + 1)
                nc.vector.tensor_reduce(
                    out=dotn[:, t : t + 1], in_=S3[t % 3][:], axis=AX, op=add
                ).then_inc(dve_red, 1)

            for g in range(NG):
                b = g % BUFS
                k = 16 * (g // BUFS + 1)
                vector.wait_ge(SEM_U[b], k)
                vector.wait_ge(SEM_V[b], k)
                for a in range(G):
                    t = g * G + a
                    if t >= 3:  # S2 slot: ACT copy of t-3 retired
                        vector.wait_ge(act_s2, t - 2)
                    nc.vector.tensor_tensor(
                        out=S2[t % 3][:], in0=U[b][:, a, :], in1=V[b][:, a, :],
                        op=mult,
                    ).then_inc(dve_s2, 1)
                    if a == 0:
                        vector.wait_ge(SEM_W[b], k)  # w loaded
                    if t >= 3:  # S3 slot: reduce of t-3 retired
                        vector.wait_ge(dve_red, t - 2)
                    nc.vector.tensor_tensor(
                        out=S3[t % 3][:], in0=U[b][:, a, :], in1=W[b][:, a, :],
                        op=mult,
                    ).then_inc(dve_s3, 1)
                    if t >= 1:
                        reduces(t - 1)
            reduces(T - 1)

        @block.scalar
        def _(scalar):
            def cp(t):
                # dotp reduce for sub-tile t (issued one sub-tile late)
                scalar.wait_ge(dve_s2, t + 1)  # product retired
                if t >= 2:
                    scalar.wait_ge(act_s2, t - 1)  # CPD slot writer retired
                nc.scalar.activation(
                    out=CPD[t % 2][:], in_=S2[t % 3][:], func=Cp,
                    accum_out=dotp[:, t : t + 1],
                ).then_inc(act_s2, 1)

            for g in range(NG):
                b = g % BUFS
                scalar.wait_ge(SEM_U[b], 16 * (g // BUFS + 1))  # u loaded
                for a in range(G):
                    t = g * G + a
                    if t >= 2:
                        scalar.wait_ge(act_sq, t - 1)  # SQD slot writer retired
                    nc.scalar.activation(
                        out=SQD[t % 2][:], in_=U[b][:, a, :], func=Sq,
                        accum_out=nu2[:, t : t + 1],
                    ).then_inc(act_sq, 1)
                    if t >= 1:
                        cp(t - 1)
            cp(T - 1)

    return nc


def kernel(embeddings, labels, pos_idx, neg_idx):
    global LAST_RESULT
    from concourse.bass_utils import run_bass_kernel_spmd

    emb = np.asarray(embeddings, dtype=np.float32).astype(np.float16)
    assert emb.shape == (N, D)
    pidx = np.asarray(pos_idx).astype(np.int64)
    nidx = np.asarray(neg_idx).astype(np.int64)

    in_maps = []
    for i in range(NCORES):
        sl = slice(i * RPC, (i + 1) * RPC)
        in_maps.append(
            {
                "anc": np.ascontiguousarray(emb[sl]),
                "pos": np.ascontiguousarray(emb[pidx[sl]]),
                "neg": np.ascontiguousarray(emb[nidx[sl]]),
            }
        )

    nc = _CACHE.get("nc")
    if nc is None:
        nc = _build_nc()
        _CACHE["nc"] = nc

    res = run_bass_kernel_spmd(nc, in_maps, list(range(NCORES)))
    LAST_RESULT = res

    # out[k] is [128, T]: row p, col t -> shard row t*128+p
    def decode(k):
        return np.concatenate(
            [res.results[i]["out"][k].T.ravel() for i in range(NCORES)]
        ).astype(np.float64)

    nu2 = decode(0)
    P = decode(1)
    Q = decode(2)

    norm = np.sqrt(nu2)
    den = np.maximum(norm, EPS)  # F.normalize clamp
    ahat2 = nu2 / (den * den)  # ||a_hat||^2 (==1 unless degenerate)

    def dist(idx, dot):
        S = ahat2 + ahat2[idx] - 2.0 * dot / (den * den[idx]) + D * EPS * EPS
        return np.sqrt(np.maximum(S, 0.0)) + EPS

    d_pos = dist(pidx, P)
    d_neg = dist(nidx, Q)
    pos_loss = d_pos * d_pos
    neg_loss = np.maximum(MARGIN - d_neg, EPS) ** 2
    total = pos_loss.sum() + neg_loss.sum()
    return np.array(total / (2.0 * N), dtype=np.float32)



# revision 3
# speedup vs baseline: 1.0030x; 1.0030x over previous
"""Trainium2 Bass kernel for ContrastiveLoss (N=16384, D=1024, 8 NeuronCores).

Strategy (data-parallel over anchors, transposed layout):
  - Host shards anchor rows across 8 cores (2048 rows each), gathers pos/neg
    rows (gather commutes with row-wise normalization), TRANSPOSES each block
    to [D, rows] = [1024, 2048] fp16 and reshapes to [8, 128, 2048] k-chunks.
  - Device per chunk c (partitions = 128 dims, free = 2048 rows):
      DVE:  P_uv = U_c * V_c, P_uw = U_c * W_c   (fp16 tensor_tensor, 2x mode)
      ACT:  P_uu = Square(U_c)                   (fp16 activation)
      PE :  ones[128,32]^T @ P_s  -> psum[32s:32s+32, 512cg:512cg+512]
            accumulated over the 8 k-chunks (partition-axis reduction at
            ~N cycles/matmul -- far faster than DVE/ACT free-axis reduces).
  - Tail: DVE copies psum -> sbuf, one DMA stores [96, 2048] f32 stats
    (rows 0/32/64 = sum u*u, sum u*v, sum u*w per anchor).
  - Host epilogue (f64) reconstructs the reference math from raw-embedding
    dots:  a = u/max(|u|,eps),  ||a-b+eps||^2 ~= ahat2_a + ahat2_b
           - 2<u,v>/(den_a den_b) + D*eps^2, then the margin loss.
"""

import sys

for _p in ("/opt/trn_rl_repo", "/root/.axon_site/_ro/trn_rl_repo"):
    if _p not in sys.path:
        sys.path.append(_p)

import numpy as np

N = 16384  # total rows
D = 1024  # embedding dim
NCORES = 8
RPC = N // NCORES  # rows per core = 2048
KC = D // 128  # k-chunks per core = 8
PSLOTS = 4  # product buffer slots per stat
NCG = RPC // 512  # 512-col groups = 4
EPS = 1e-6
MARGIN = 1.0

LAST_RESULT = None
_CACHE = {}


def _build_nc():
    import concourse.bass as bass
    import concourse.mybir as mybir

    f32 = mybir.dt.float32
    f16 = mybir.dt.float16
    nc = bass.Bass()
    # transposed chunked inputs: [chunk, 128 dims, 2048 rows]
    anc = nc.declare_dram_parameter("anc", [KC, 128, RPC], f16, isOutput=False)
    pos = nc.declare_dram_parameter("pos", [KC, 128, RPC], f16, isOutput=False)
    neg = nc.declare_dram_parameter("neg", [KC, 128, RPC], f16, isOutput=False)
    one = nc.declare_dram_parameter("one", [128, 32], f16, isOutput=False)
    out = nc.declare_dram_parameter("out", [96, RPC], f32, isOutput=True)

    Sq = mybir.ActivationFunctionType.Square
    mult = mybir.AluOpType.mult

    from contextlib import ExitStack

    with ExitStack() as ctx:
        sb = lambda nm, shape, dt: ctx.enter_context(nc.sbuf_tensor(nm, shape, dt))
        ps = lambda nm, shape, dt: ctx.enter_context(nc.psum_tensor(nm, shape, dt))
        sem = lambda nm: ctx.enter_context(nc.semaphore(nm))

        U = sb("u", [128, KC, RPC], f16)
        V = sb("v", [128, KC, RPC], f16)
        W = sb("w", [128, KC, RPC], f16)
        ONES = sb("ones", [128, 32], f16)
        # product slots: [128, PSLOTS, 2048] per stat
        PUV = sb("puv", [128, PSLOTS, RPC], f16)
        PUW = sb("puw", [128, PSLOTS, RPC], f16)
        PUU = sb("puu", [128, PSLOTS, RPC], f16)
        STATS = sb("stats", [96, RPC], f32)
        PS = ps("ps", [96, RPC], f32)  # banks 0-3; rows 0-31 uu, 32-63 uv, 64-95 uw

        # load sems: one per DMA so completion order is unambiguous
        SU = [sem(f"su{c}") for c in range(KC)]
        SV = [sem(f"sv{c}") for c in range(KC)]
        SW = [sem(f"sw{c}") for c in range(KC)]
        s_one = sem("s_one")
        dve_uv = sem("dve_uv")  # +1 per finished TT uv
        dve_uw = sem("dve_uw")
        act_sq = sem("act_sq")  # +1 per finished Square
        pe_sem = sem("pe_sem")  # +1 per finished chunk of 12 matmuls
        ext_sem = sem("ext_sem")
        st_sem = sem("st_sem")

        block = ctx.enter_context(nc.Block())

        @block.sync
        def _(sync):
            sync.dma_start(out=ONES[:], in_=one[:, :]).then_inc(s_one, 16)
            for c in range(KC):
                sync.dma_start(out=U[:, c, :], in_=anc[c]).then_inc(SU[c], 16)
                sync.dma_start(out=V[:, c, :], in_=pos[c]).then_inc(SV[c], 16)
                sync.dma_start(out=W[:, c, :], in_=neg[c]).then_inc(SW[c], 16)
            sync.wait_ge(ext_sem, 1)
            sync.dma_start(out=out[:, :], in_=STATS[:]).then_inc(st_sem, 16)
            sync.wait_ge(st_sem, 16)

        @block.vector
        def _(vector):
            for c in range(KC):
                sl = c % PSLOTS
                if c >= PSLOTS:
                    vector.wait_ge(pe_sem, c - PSLOTS + 1)
                vector.wait_ge(SU[c], 16)
                vector.wait_ge(SV[c], 16)
                nc.vector.tensor_tensor(
                    out=PUV[:, sl, :], in0=U[:, c, :], in1=V[:, c, :], op=mult
                ).then_inc(dve_uv, 1)
                vector.wait_ge(SW[c], 16)
                nc.vector.tensor_tensor(
                    out=PUW[:, sl, :], in0=U[:, c, :], in1=W[:, c, :], op=mult
                ).then_inc(dve_uw, 1)
            # extract accumulated stats from psum once all matmuls retired
            vector.wait_ge(pe_sem, KC)
            nc.vector.tensor_copy(out=STATS[:], in_=PS[:]).then_inc(ext_sem, 1)

        @block.scalar
        def _(scalar):
            for c in range(KC):
                sl = c % PSLOTS
                if c >= PSLOTS:
                    scalar.wait_ge(pe_sem, c - PSLOTS + 1)
                scalar.wait_ge(SU[c], 16)
                nc.scalar.activation(
                    out=PUU[:, sl, :], in_=U[:, c, :], func=Sq
                ).then_inc(act_sq, 1)

        @block.tensor
        def _(tensor):
            tensor.wait_ge(s_one, 16)
            for c in range(KC):
                sl = c % PSLOTS
                tensor.wait_ge(act_sq, c + 1)
                tensor.wait_ge(dve_uv, c + 1)
                tensor.wait_ge(dve_uw, c + 1)
                for cg in range(NCG):
                    co = 512 * cg
                    for s, P in enumerate((PUU, PUV, PUW)):
                        mm = nc.tensor.matmul(
                            out=PS[32 * s : 32 * s + 32, co : co + 512],
                            lhsT=ONES[:, :],
                            rhs=P[:, sl, co : co + 512],
                            start=(c == 0),
                            stop=(c == KC - 1),
                        )
                        if cg == NCG - 1 and s == 2:
                            # matmuls retire in pc order: one inc on the last
                            # matmul of the chunk covers all 12
                            mm.then_inc(pe_sem, 1)

    return nc


def kernel(embeddings, labels, pos_idx, neg_idx):
    global LAST_RESULT
    from concourse.bass_utils import run_bass_kernel_spmd

    emb = np.asarray(embeddings, dtype=np.float32).astype(np.float16)
    assert emb.shape == (N, D)
    pidx = np.asarray(pos_idx).astype(np.int64)
    nidx = np.asarray(neg_idx).astype(np.int64)
    ones = np.ones((128, 32), dtype=np.float16)

    def tchunks(rows):
        # [2048, 1024] -> [1024, 2048] -> [8, 128, 2048]
        return np.ascontiguousarray(rows.T).reshape(KC, 128, RPC)

    in_maps = []
    for i in range(NCORES):
        sl = slice(i * RPC, (i + 1) * RPC)
        in_maps.append(
            {
                "anc": tchunks(emb[sl]),
                "pos": tchunks(emb[pidx[sl]]),
                "neg": tchunks(emb[nidx[sl]]),
                "one": ones,
            }
        )

    nc = _CACHE.get("nc")
    if nc is None:
        nc = _build_nc()
        _CACHE["nc"] = nc

    res = run_bass_kernel_spmd(nc, in_maps, list(range(NCORES)))
    LAST_RESULT = res

    def decode(k):
        # stat row k*32 of [96, 2048] output, concatenated across cores
        return np.concatenate(
            [res.results[i]["out"][32 * k] for i in range(NCORES)]
        ).astype(np.float64)

    nu2 = decode(0)
    P = decode(1)
    Q = decode(2)

    norm = np.sqrt(nu2)
    den = np.maximum(norm, EPS)  # F.normalize clamp
    ahat2 = nu2 / (den * den)  # ||a_hat||^2 (==1 unless degenerate)

    def dist(idx, dot):
        S = ahat2 + ahat2[idx] - 2.0 * dot / (den * den[idx]) + D * EPS * EPS
        return np.sqrt(np.maximum(S, 0.0)) + EPS

    d_pos = dist(pidx, P)
    d_neg = dist(nidx, Q)
    pos_loss = d_pos * d_pos
    neg_loss = np.maximum(MARGIN - d_neg, EPS) ** 2
    total = pos_loss.sum() + neg_loss.sum()
    return np.array(total / (2.0 * N), dtype=np.float32)


# revision 4
# speedup vs baseline: 1.1760x; 1.1725x over previous
"""Trainium2 Bass kernel for ContrastiveLoss (N=16384, D=1024, 8 NeuronCores).

Strategy (data-parallel over anchors, transposed layout, fp8 HBM traffic):
  - Host shards anchor rows across 8 cores (2048 rows each), gathers pos/neg
    rows (gather commutes with row-wise normalization), casts to fp8e4,
    TRANSPOSES each block to [D, rows] and reshapes to [8, 128, 2048] chunks.
  - Device loads via SWDGE cast-DMAs (fp8 HBM -> fp16 SBUF): HBM reads are
    halved; the SBUF write side binds at the DMA-fabric rate.
  - Per chunk c (partitions = 128 dims, free = 2048 rows):
      DVE:  P_uv = U_c * V_c, P_uw = U_c * W_c   (fp16 tensor_tensor, 2x mode)
      ACT:  P_uu = Square(U_c)                   (fp16 activation)
      PE :  ones[128,32]^T @ P_s  -> psum[32s:32s+32, 512cg:512cg+512]
            accumulated over the 8 k-chunks (partition-axis reduction at
            ~N cycles/matmul -- far faster than DVE/ACT free-axis reduces).
  - Tail is fine-grained: last neg chunk is loaded in column halves, uu/uv
    stats are extracted while the uw tail still runs, and the store is a
    24 KB partition-strided DMA of rows {0,32,64}.
  - Host epilogue (f64) reconstructs the reference math from raw-embedding
    dots:  a = u/max(|u|,eps),  ||a-b+eps||^2 ~= ahat2_a + ahat2_b
           - 2<u,v>/(den_a den_b) + D*eps^2, then the margin loss.
"""

import sys

for _p in ("/opt/trn_rl_repo", "/root/.axon_site/_ro/trn_rl_repo"):
    if _p not in sys.path:
        sys.path.append(_p)

import numpy as np
import ml_dtypes

N = 16384  # total rows
D = 1024  # embedding dim
NCORES = 8
RPC = N // NCORES  # rows per core = 2048
KC = D // 128  # k-chunks per core = 8
PSLOTS = 4  # product buffer slots per stat
NCG = RPC // 512  # 512-col matmul groups = 4
EPS = 1e-6
MARGIN = 1.0

LAST_RESULT = None
_CACHE = {}


def _build_nc():
    import concourse.bass as bass
    import concourse.mybir as mybir

    f32 = mybir.dt.float32
    f16 = mybir.dt.float16
    fp8 = mybir.dt.float8e4
    nc = bass.Bass()
    # transposed chunked inputs: [chunk, 128 dims, 2048 rows] in fp8
    anc = nc.declare_dram_parameter("anc", [KC, 128, RPC], fp8, isOutput=False)
    pos = nc.declare_dram_parameter("pos", [KC, 128, RPC], fp8, isOutput=False)
    neg = nc.declare_dram_parameter("neg", [KC, 128, RPC], fp8, isOutput=False)
    one = nc.declare_dram_parameter("one", [128, 32], f16, isOutput=False)
    out = nc.declare_dram_parameter("out", [3, RPC], f32, isOutput=True)

    Sq = mybir.ActivationFunctionType.Square
    mult = mybir.AluOpType.mult

    from contextlib import ExitStack

    with ExitStack() as ctx:
        sb = lambda nm, shape, dt: ctx.enter_context(nc.sbuf_tensor(nm, shape, dt))
        ps_ = lambda nm, shape, dt: ctx.enter_context(nc.psum_tensor(nm, shape, dt))
        sem = lambda nm: ctx.enter_context(nc.semaphore(nm))

        U = sb("u", [128, KC, RPC], f16)
        V = sb("v", [128, KC, RPC], f16)
        W = sb("w", [128, KC, RPC], f16)
        ONES = sb("ones", [128, 32], f16)
        PUV = sb("puv", [128, PSLOTS, RPC], f16)
        PUW = sb("puw", [128, PSLOTS, RPC], f16)
        PUU = sb("puu", [128, PSLOTS, RPC], f16)
        STATS = sb("stats", [96, RPC], f32)
        PS = ps_("ps", [96, RPC], f32)  # rows 0-31 uu, 32-63 uv, 64-95 uw

        SU = [sem(f"su{c}") for c in range(KC)]
        SV = [sem(f"sv{c}") for c in range(KC)]
        SW = [sem(f"sw{c}") for c in range(KC)]  # SW[KC-1] used for half a
        sw_b = sem("sw_b")  # last neg chunk, cols 1024:2048
        s_one = sem("s_one")
        dve_uv = sem("dve_uv")  # +1 per TT uv
        dve_uw = sem("dve_uw")  # +1 per TT uw (9 total: last chunk split)
        act_sq = sem("act_sq")  # +1 per Square
        pe_uu = sem("pe_uu")  # +1 per finished (uu, chunk) matmul group
        pe_uv = sem("pe_uv")
        pe_uw = sem("pe_uw")  # 9 total: last chunk in two half-groups
        ext_sem = sem("ext_sem")
        st_sem = sem("st_sem")

        block = ctx.enter_context(nc.Block())

        H = RPC // 2  # column half

        @block.sync
        def _(sync):
            sync.dma_start(out=ONES[:], in_=one[:, :]).then_inc(s_one, 16)
            sync.wait_ge(ext_sem, 3)
            sync.dma_start(out=out[:, :], in_=STATS[0:96:32, :]).then_inc(st_sem, 16)
            sync.wait_ge(st_sem, 16)

        @block.gpsimd
        def _(g):
            # SWDGE cast loads fp8 -> fp16; last neg chunk in column halves
            for c in range(KC):
                g.dma_start(out=U[:, c, :], in_=anc[c]).then_inc(SU[c], 16)
                g.dma_start(out=V[:, c, :], in_=pos[c]).then_inc(SV[c], 16)
                if c < KC - 1:
                    g.dma_start(out=W[:, c, :], in_=neg[c]).then_inc(SW[c], 16)
                else:
                    g.dma_start(out=W[:, c, 0:H], in_=neg[c][:, 0:H]).then_inc(
                        SW[c], 16
                    )
                    g.dma_start(out=W[:, c, H:RPC], in_=neg[c][:, H:RPC]).then_inc(
                        sw_b, 16
                    )

        @block.vector
        def _(vector):
            for c in range(KC):
                sl = c % PSLOTS
                if c >= PSLOTS:
                    vector.wait_ge(pe_uv, c - PSLOTS + 1)
                vector.wait_ge(SU[c], 16)
                vector.wait_ge(SV[c], 16)
                nc.vector.tensor_tensor(
                    out=PUV[:, sl, :], in0=U[:, c, :], in1=V[:, c, :], op=mult
                ).then_inc(dve_uv, 1)
                if c >= PSLOTS:
                    vector.wait_ge(pe_uw, c - PSLOTS + 1)
                vector.wait_ge(SW[c], 16)
                if c < KC - 1:
                    nc.vector.tensor_tensor(
                        out=PUW[:, sl, :], in0=U[:, c, :], in1=W[:, c, :], op=mult
                    ).then_inc(dve_uw, 1)
                else:
                    nc.vector.tensor_tensor(
                        out=PUW[:, sl, 0:H], in0=U[:, c, 0:H], in1=W[:, c, 0:H],
                        op=mult,
                    ).then_inc(dve_uw, 1)
                    vector.wait_ge(sw_b, 16)
                    nc.vector.tensor_tensor(
                        out=PUW[:, sl, H:RPC], in0=U[:, c, H:RPC],
                        in1=W[:, c, H:RPC], op=mult,
                    ).then_inc(dve_uw, 1)

        @block.scalar
        def _(scalar):
            for c in range(KC):
                sl = c % PSLOTS
                if c >= PSLOTS:
                    scalar.wait_ge(pe_uu, c - PSLOTS + 1)
                scalar.wait_ge(SU[c], 16)
                nc.scalar.activation(
                    out=PUU[:, sl, :], in_=U[:, c, :], func=Sq
                ).then_inc(act_sq, 1)
            # extraction: uu+uv rows while the uw tail still runs, then uw
            scalar.wait_ge(pe_uu, KC)
            scalar.wait_ge(pe_uv, KC)
            nc.scalar.copy(out=STATS[0:64, :], in_=PS[0:64, :]).then_inc(ext_sem, 1)
            scalar.wait_ge(pe_uw, KC)  # uw cols 0:H accumulated
            nc.scalar.copy(out=STATS[64:96, 0:H], in_=PS[64:96, 0:H]).then_inc(
                ext_sem, 1
            )
            scalar.wait_ge(pe_uw, KC + 1)
            nc.scalar.copy(out=STATS[64:96, H:RPC], in_=PS[64:96, H:RPC]).then_inc(
                ext_sem, 1
            )

        @block.tensor
        def _(tensor):
            def mms(P, sl, s, cgs, c):
                for i, cg in enumerate(cgs):
                    co = 512 * cg
                    mm = nc.tensor.matmul(
                        out=PS[32 * s : 32 * s + 32, co : co + 512],
                        lhsT=ONES[:, :],
                        rhs=P[:, sl, co : co + 512],
                        start=(c == 0),
                        stop=(c == KC - 1),
                    )
                    if i == len(cgs) - 1:
                        mm.then_inc((pe_uu, pe_uv, pe_uw)[s], 1)

            tensor.wait_ge(s_one, 16)
            for c in range(KC):
                sl = c % PSLOTS
                tensor.wait_ge(act_sq, c + 1)
                mms(PUU, sl, 0, range(NCG), c)
                tensor.wait_ge(dve_uv, c + 1)
                mms(PUV, sl, 1, range(NCG), c)
                if c < KC - 1:
                    tensor.wait_ge(dve_uw, c + 1)
                    mms(PUW, sl, 2, range(NCG), c)
                else:
                    tensor.wait_ge(dve_uw, KC)
                    mms(PUW, sl, 2, range(NCG // 2), c)
                    tensor.wait_ge(dve_uw, KC + 1)
                    mms(PUW, sl, 2, range(NCG // 2, NCG), c)

    return nc


def kernel(embeddings, labels, pos_idx, neg_idx):
    global LAST_RESULT
    from concourse.bass_utils import run_bass_kernel_spmd

    emb = np.asarray(embeddings, dtype=np.float32).astype(ml_dtypes.float8_e4m3)
    assert emb.shape == (N, D)
    pidx = np.asarray(pos_idx).astype(np.int64)
    nidx = np.asarray(neg_idx).astype(np.int64)
    ones = np.ones((128, 32), dtype=np.float16)

    def tchunks(rows):
        # [2048, 1024] -> [1024, 2048] -> [8, 128, 2048]
        return np.ascontiguousarray(rows.T).reshape(KC, 128, RPC)

    in_maps = []
    for i in range(NCORES):
        sl = slice(i * RPC, (i + 1) * RPC)
        in_maps.append(
            {
                "anc": tchunks(emb[sl]),
                "pos": tchunks(emb[pidx[sl]]),
                "neg": tchunks(emb[nidx[sl]]),
                "one": ones,
            }
        )

    nc = _CACHE.get("nc")
    if nc is None:
        nc = _build_nc()
        _CACHE["nc"] = nc

    res = run_bass_kernel_spmd(nc, in_maps, list(range(NCORES)))
    LAST_RESULT = res

    def decode(k):
        # stat row k of the [3, 2048] output, concatenated across cores
        return np.concatenate(
            [res.results[i]["out"][k] for i in range(NCORES)]
        ).astype(np.float64)

    nu2 = decode(0)
    P = decode(1)
    Q = decode(2)

    norm = np.sqrt(nu2)
    den = np.maximum(norm, EPS)  # F.normalize clamp
    ahat2 = nu2 / (den * den)  # ||a_hat||^2 (==1 unless degenerate)

    def dist(idx, dot):
        S = ahat2 + ahat2[idx] - 2.0 * dot / (den * den[idx]) + D * EPS * EPS
        return np.sqrt(np.maximum(S, 0.0)) + EPS

    d_pos = dist(pidx, P)
    d_neg = dist(nidx, Q)
    pos_loss = d_pos * d_pos
    neg_loss = np.maximum(MARGIN - d_neg, EPS) ** 2
    total = pos_loss.sum() + neg_loss.sum()
    return np.array(total / (2.0 * N), dtype=np.float32)


# revision 5
# speedup vs baseline: 1.2859x; 1.0935x over previous
"""Trainium2 Bass kernel for ContrastiveLoss (N=16384, D=1024, 8 NeuronCores).

Strategy (data-parallel over anchors, transposed layout, fp8 HBM traffic):
  - Host shards anchor rows across 8 cores (2048 rows each), gathers pos/neg
    rows (gather commutes with row-wise normalization), casts to fp8e4,
    TRANSPOSES each block to [D, rows] = chunks of [128, 2048].
  - Device loads via SWDGE cast-DMAs (fp8 HBM -> fp16 SBUF): HBM reads are
    halved; the SBUF write side binds at the DMA-fabric rate. Loads are
    issued before the block barrier, pair-batched mid-stream, and split
    into column halves for the last chunk to shorten the tail.
  - Per chunk c (partitions = 128 dims, free = 2048 rows):
      DVE:  P_uv = U_c * V_c, P_uw = U_c * W_c   (fp16 tensor_tensor, 2x mode)
      ACT:  P_uu = Square(U_c)                   (fp16 activation)
      PE :  ones[128,32]^T @ P_s  -> psum[32s:32s+32, 512cg:512cg+512]
            accumulated over the 8 k-chunks (partition-axis reduction at
            ~N cycles/matmul -- far faster than DVE/ACT free-axis reduces).
  - Tail: the last chunk runs column-half granular; stats are extracted
    (ACT psum->sbuf copies) and stored (partition-strided 12 KB DMAs)
    per half as soon as their matmul groups retire.
  - Host epilogue (f64) reconstructs the reference math from raw-embedding
    dots:  a = u/max(|u|,eps),  ||a-b+eps||^2 ~= ahat2_a + ahat2_b
           - 2<u,v>/(den_a den_b) + D*eps^2, then the margin loss.
"""

import sys

for _p in ("/opt/trn_rl_repo", "/root/.axon_site/_ro/trn_rl_repo"):
    if _p not in sys.path:
        sys.path.append(_p)

import numpy as np
import ml_dtypes

N = 16384  # total rows
D = 1024  # embedding dim
NCORES = 8
RPC = N // NCORES  # rows per core = 2048
KC = D // 128  # k-chunks per core = 8
NPAIR = 3  # chunk pairs 0-5 loaded as [128, 2, 2048] DMAs
PSLOTS = 4  # product buffer slots per stat
NCG = RPC // 512  # 512-col matmul groups = 4
H = RPC // 2  # column half
EPS = 1e-6
MARGIN = 1.0

LAST_RESULT = None
_CACHE = {}


def _build_nc():
    import concourse.bass as bass
    import concourse.mybir as mybir

    f32 = mybir.dt.float32
    f16 = mybir.dt.float16
    fp8 = mybir.dt.float8e4
    nc = bass.Bass()
    ancp = nc.declare_dram_parameter("ancp", [NPAIR, 128, 2, RPC], fp8, isOutput=False)
    posp = nc.declare_dram_parameter("posp", [NPAIR, 128, 2, RPC], fp8, isOutput=False)
    negp = nc.declare_dram_parameter("negp", [NPAIR, 128, 2, RPC], fp8, isOutput=False)
    anc2 = nc.declare_dram_parameter("anc2", [2, 128, RPC], fp8, isOutput=False)
    pos2 = nc.declare_dram_parameter("pos2", [2, 128, RPC], fp8, isOutput=False)
    neg2 = nc.declare_dram_parameter("neg2", [2, 128, RPC], fp8, isOutput=False)
    one = nc.declare_dram_parameter("one", [128, 32], f16, isOutput=False)
    out = nc.declare_dram_parameter("out", [3, RPC], f32, isOutput=True)

    Sq = mybir.ActivationFunctionType.Square
    mult = mybir.AluOpType.mult

    from contextlib import ExitStack

    with ExitStack() as ctx:
        sb = lambda nm, shape, dt: ctx.enter_context(nc.sbuf_tensor(nm, shape, dt))
        ps_ = lambda nm, shape, dt: ctx.enter_context(nc.psum_tensor(nm, shape, dt))
        sem = lambda nm: ctx.enter_context(nc.semaphore(nm))

        U = sb("u", [128, KC, RPC], f16)
        V = sb("v", [128, KC, RPC], f16)
        W = sb("w", [128, KC, RPC], f16)
        ONES = sb("ones", [128, 32], f16)
        PUV = sb("puv", [128, PSLOTS, RPC], f16)
        PUW = sb("puw", [128, PSLOTS, RPC], f16)
        PUU = sb("puu", [128, PSLOTS, RPC], f16)
        STATS = sb("stats", [96, RPC], f32)
        PS = ps_("ps", [96, RPC], f32)  # rows 0-31 uu, 32-63 uv, 64-95 uw

        SUP = [sem(f"sup{p}") for p in range(NPAIR)]
        SVP = [sem(f"svp{p}") for p in range(NPAIR)]
        SWP = [sem(f"swp{p}") for p in range(NPAIR)]
        SU6, SV6, SW6 = sem("su6"), sem("sv6"), sem("sw6")
        SU7 = [sem(f"su7{h}") for h in range(2)]
        SV7 = [sem(f"sv7{h}") for h in range(2)]
        SW7 = [sem(f"sw7{h}") for h in range(2)]
        s_one = sem("s_one")
        dve_uv = sem("dve_uv")  # +1 per TT uv, chunks 0-6
        dve_uw = sem("dve_uw")
        act_sq = sem("act_sq")  # +1 per Square, chunks 0-6
        d7_uv = sem("d7_uv")  # chunk-7 half TTs
        d7_uw = sem("d7_uw")
        a7_sq = sem("a7_sq")
        pe_uu = sem("pe_uu")  # +1 per finished (stat, chunk) group, chunks 0-6
        pe_uv = sem("pe_uv")
        pe_uw = sem("pe_uw")
        pe_h7 = sem("pe_h7")  # +1 per finished chunk-7 column-half
        ext_sem = sem("ext_sem")
        st_sem = sem("st_sem")

        # ---- loads issued before the block barrier ----
        nc.sync.dma_start(out=ONES[:], in_=one[:, :]).then_inc(s_one, 16)
        for p in range(NPAIR):
            nc.gpsimd.dma_start(out=U[:, 2 * p : 2 * p + 2, :], in_=ancp[p]).then_inc(
                SUP[p], 16
            )
            nc.gpsimd.dma_start(out=V[:, 2 * p : 2 * p + 2, :], in_=posp[p]).then_inc(
                SVP[p], 16
            )
            nc.gpsimd.dma_start(out=W[:, 2 * p : 2 * p + 2, :], in_=negp[p]).then_inc(
                SWP[p], 16
            )
        nc.gpsimd.dma_start(out=U[:, 6, :], in_=anc2[0]).then_inc(SU6, 16)
        nc.gpsimd.dma_start(out=V[:, 6, :], in_=pos2[0]).then_inc(SV6, 16)
        nc.gpsimd.dma_start(out=W[:, 6, :], in_=neg2[0]).then_inc(SW6, 16)
        for h in range(2):
            cs = slice(H * h, H * h + H)
            nc.gpsimd.dma_start(out=U[:, 7, cs], in_=anc2[1][:, cs]).then_inc(
                SU7[h], 16
            )
            nc.gpsimd.dma_start(out=V[:, 7, cs], in_=pos2[1][:, cs]).then_inc(
                SV7[h], 16
            )
            nc.gpsimd.dma_start(out=W[:, 7, cs], in_=neg2[1][:, cs]).then_inc(
                SW7[h], 16
            )

        def u_wait(eng, c):
            if c < 6:
                eng.wait_ge(SUP[c // 2], 16)
            elif c == 6:
                eng.wait_ge(SU6, 16)

        def v_wait(eng, c):
            if c < 6:
                eng.wait_ge(SVP[c // 2], 16)
            elif c == 6:
                eng.wait_ge(SV6, 16)

        def w_wait(eng, c):
            if c < 6:
                eng.wait_ge(SWP[c // 2], 16)
            elif c == 6:
                eng.wait_ge(SW6, 16)

        block = ctx.enter_context(nc.Block())

        @block.sync
        def _(sync):
            for h in range(2):
                cs = slice(H * h, H * h + H)
                sync.wait_ge(ext_sem, h + 1)
                sync.dma_start(out=out[:, cs], in_=STATS[0:96:32, cs]).then_inc(
                    st_sem, 16
                )
            sync.wait_ge(st_sem, 32)

        @block.vector
        def _(vector):
            for c in range(7):
                sl = c % PSLOTS
                if c >= PSLOTS:
                    vector.wait_ge(pe_uv, c - PSLOTS + 1)
                u_wait(vector, c)
                v_wait(vector, c)
                nc.vector.tensor_tensor(
                    out=PUV[:, sl, :], in0=U[:, c, :], in1=V[:, c, :], op=mult
                ).then_inc(dve_uv, 1)
                if c >= PSLOTS:
                    vector.wait_ge(pe_uw, c - PSLOTS + 1)
                w_wait(vector, c)
                nc.vector.tensor_tensor(
                    out=PUW[:, sl, :], in0=U[:, c, :], in1=W[:, c, :], op=mult
                ).then_inc(dve_uw, 1)
            # chunk 7, column-half granular (slot 3)
            vector.wait_ge(pe_uv, 4)
            vector.wait_ge(pe_uw, 4)
            for h in range(2):
                cs = slice(H * h, H * h + H)
                vector.wait_ge(SU7[h], 16)
                vector.wait_ge(SV7[h], 16)
                nc.vector.tensor_tensor(
                    out=PUV[:, 3, cs], in0=U[:, 7, cs], in1=V[:, 7, cs], op=mult
                ).then_inc(d7_uv, 1)
                vector.wait_ge(SW7[h], 16)
                nc.vector.tensor_tensor(
                    out=PUW[:, 3, cs], in0=U[:, 7, cs], in1=W[:, 7, cs], op=mult
                ).then_inc(d7_uw, 1)

        @block.scalar
        def _(scalar):
            for c in range(7):
                sl = c % PSLOTS
                if c >= PSLOTS:
                    scalar.wait_ge(pe_uu, c - PSLOTS + 1)
                u_wait(scalar, c)
                nc.scalar.activation(
                    out=PUU[:, sl, :], in_=U[:, c, :], func=Sq
                ).then_inc(act_sq, 1)
            scalar.wait_ge(pe_uu, 4)
            for h in range(2):
                cs = slice(H * h, H * h + H)
                scalar.wait_ge(SU7[h], 16)
                nc.scalar.activation(
                    out=PUU[:, 3, cs], in_=U[:, 7, cs], func=Sq
                ).then_inc(a7_sq, 1)
            # per-half extraction once that half's matmul groups retired
            for h in range(2):
                cs = slice(H * h, H * h + H)
                scalar.wait_ge(pe_h7, h + 1)
                nc.scalar.copy(out=STATS[0:96, cs], in_=PS[0:96, cs]).then_inc(
                    ext_sem, 1
                )

        @block.tensor
        def _(tensor):
            sems = (pe_uu, pe_uv, pe_uw)

            def mms(P, sl, s, cgs, c, inc=None):
                for i, cg in enumerate(cgs):
                    co = 512 * cg
                    mm = nc.tensor.matmul(
                        out=PS[32 * s : 32 * s + 32, co : co + 512],
                        lhsT=ONES[:, :],
                        rhs=P[:, sl, co : co + 512],
                        start=(c == 0),
                        stop=(c == KC - 1),
                    )
                    if i == len(cgs) - 1 and inc is not None:
                        mm.then_inc(inc, 1)

            tensor.wait_ge(s_one, 16)
            for c in range(7):
                sl = c % PSLOTS
                tensor.wait_ge(act_sq, c + 1)
                mms(PUU, sl, 0, range(NCG), c, pe_uu)
                tensor.wait_ge(dve_uv, c + 1)
                mms(PUV, sl, 1, range(NCG), c, pe_uv)
                tensor.wait_ge(dve_uw, c + 1)
                mms(PUW, sl, 2, range(NCG), c, pe_uw)
            for h in range(2):
                cgs = (2 * h, 2 * h + 1)
                tensor.wait_ge(a7_sq, h + 1)
                mms(PUU, 3, 0, cgs, 7)
                tensor.wait_ge(d7_uv, h + 1)
                mms(PUV, 3, 1, cgs, 7)
                tensor.wait_ge(d7_uw, h + 1)
                mms(PUW, 3, 2, cgs, 7, pe_h7)

    return nc


def kernel(embeddings, labels, pos_idx, neg_idx):
    global LAST_RESULT
    from concourse.bass_utils import run_bass_kernel_spmd

    emb = np.asarray(embeddings, dtype=np.float32).astype(ml_dtypes.float8_e4m3)
    assert emb.shape == (N, D)
    pidx = np.asarray(pos_idx).astype(np.int64)
    nidx = np.asarray(neg_idx).astype(np.int64)
    ones = np.ones((128, 32), dtype=np.float16)

    def tchunks(rows):
        # [2048, 1024] -> chunks [8, 128, 2048]; pairs + last two chunks
        t = np.ascontiguousarray(rows.T).reshape(KC, 128, RPC)
        pairs = np.ascontiguousarray(
            t[:6].reshape(NPAIR, 2, 128, RPC).transpose(0, 2, 1, 3)
        )
        tail = np.ascontiguousarray(t[6:8])
        return pairs, tail

    in_maps = []
    for i in range(NCORES):
        sl = slice(i * RPC, (i + 1) * RPC)
        ap, a2 = tchunks(emb[sl])
        pp, p2 = tchunks(emb[pidx[sl]])
        np_, n2 = tchunks(emb[nidx[sl]])
        in_maps.append(
            {
                "ancp": ap, "anc2": a2,
                "posp": pp, "pos2": p2,
                "negp": np_, "neg2": n2,
                "one": ones,
            }
        )

    nc = _CACHE.get("nc")
    if nc is None:
        nc = _build_nc()
        _CACHE["nc"] = nc

    res = run_bass_kernel_spmd(nc, in_maps, list(range(NCORES)))
    LAST_RESULT = res

    def decode(k):
        return np.concatenate(
            [res.results[i]["out"][k] for i in range(NCORES)]
        ).astype(np.float64)

    nu2 = decode(0)
    P = decode(1)
    Q = decode(2)

    norm = np.sqrt(nu2)
    den = np.maximum(norm, EPS)  # F.normalize clamp
    ahat2 = nu2 / (den * den)  # ||a_hat||^2 (==1 unless degenerate)

    def dist(idx, dot):
        S = ahat2 + ahat2[idx] - 2.0 * dot / (den * den[idx]) + D * EPS * EPS
        return np.sqrt(np.maximum(S, 0.0)) + EPS

    d_pos = dist(pidx, P)
    d_neg = dist(nidx, Q)
    pos_loss = d_pos * d_pos
    neg_loss = np.maximum(MARGIN - d_neg, EPS) ** 2
    total = pos_loss.sum() + neg_loss.sum()
    return np.array(total / (2.0 * N), dtype=np.float32)
